# revision 8
# baseline (speedup 1.0000x reference)
"""BiLSTM + two linear heads + two CRF losses + two Viterbi decodes on 8 TRN2 cores.

Device (per core, data-parallel over batch, 64 seqs/core):
  phase A: xg[d] = x @ w_ih[d].T + b  (hi/lo-split bf16 matmuls, fp32 accum)
  phase B: 128-step LSTM recurrence, fwd+bwd interleaved, both directions
           packed on the partition axis for the elementwise chain
  phase C: logitsT[14, tok] = [w_cls; w_se] @ [h_f; h_b]
Host: CRF forward (log-space, vectorized) + Viterbi + loss assembly from logits.
"""

import numpy as np

B, S, D, H, L, LSE = 512, 128, 300, 256, 9, 5
NCORES, BC = 8, 64
TOK = S * BC          # tokens per core, col index = t*BC + b
DP = 384              # D padded: 300 data rows + 1 ones row (bias) + zero pad
G4 = 4 * H            # 1024 gate dims
NH = L + LSE          # 14 combined head classes

_CACHE = {}


def _split_hilo(x):
    import ml_dtypes
    x = np.asarray(x, np.float32)
    hi = x.astype(ml_dtypes.bfloat16)
    lo = (x - hi.astype(np.float32)).astype(ml_dtypes.bfloat16)
    return hi, lo


def _build():
    from contextlib import ExitStack
    import concourse.bass as bass
    import concourse.tile as tile
    from concourse import bacc, mybir
    from concourse.masks import make_identity

    f32 = mybir.dt.float32
    bf16 = mybir.dt.bfloat16
    AL = mybir.AluOpType
    AF = mybir.ActivationFunctionType

    nc = bacc.Bacc()
    dp = nc.declare_dram_parameter
    xT = dp("xT", [DP, TOK], f32, isOutput=False)
    wih = {}
    whh = {}
    for d in ("f", "b"):
        for p in ("hi", "lo"):
            wih[d, p] = dp(f"wih_{d}_{p}", [DP, G4], bf16, isOutput=False)
            whh[d, p] = dp(f"whh_{d}_{p}", [H, G4], bf16, isOutput=False)
    whead = {p: dp(f"whead_{p}", [2 * H, NH], bf16, isOutput=False) for p in ("hi", "lo")}
    bhead = dp("bhead", [NH, 1], f32, isOutput=False)
    logitsT = dp("logitsT", [NH, TOK], f32, isOutput=True)

    # DRAM scratch
    xg = {d: nc.dram_tensor(f"xg_{d}", [TOK, G4], f32) for d in ("f", "b")}
    hsT = {}
    for d in ("f", "b"):
        for p in ("hi", "lo"):
            hsT[d, p] = nc.dram_tensor(f"hsT_{d}_{p}", [2, 128, TOK], bf16)

    NT = TOK // 128       # 64 token-tiles per direction in phase A
    NC_CH = TOK // 512    # 16 head chunks in phase C

    with tile.TileContext(nc) as tc:
        with ExitStack() as ctx:
            wpool = ctx.enter_context(tc.tile_pool(name="weights", bufs=1))
            apool = ctx.enter_context(tc.tile_pool(name="aphase", bufs=3))
            apsum = ctx.enter_context(tc.tile_pool(name="apsum", bufs=2, space="PSUM"))
            bpool = ctx.enter_context(tc.tile_pool(name="bphase", bufs=3))
            hcpool = ctx.enter_context(tc.tile_pool(name="hc", bufs=2))
            gpsum = ctx.enter_context(tc.tile_pool(name="gpsum", bufs=2, space="PSUM"))
            tpsum = ctx.enter_context(tc.tile_pool(name="tpsum", bufs=1, space="PSUM"))
            cpool = ctx.enter_context(tc.tile_pool(name="cphase", bufs=3))
            cpsum = ctx.enter_context(tc.tile_pool(name="cpsum", bufs=1, space="PSUM"))

            # ---- load weights once
            wih_sb = {}
            whh_sb = {}
            for d in ("f", "b"):
                for p in ("hi", "lo"):
                    t = wpool.tile([128, 3, G4], bf16, tag=f"wih{d}{p}")
                    nc.sync.dma_start(
                        t[:], wih[d, p].ap().rearrange("(k p) g -> p k g", p=128)
                    )
                    wih_sb[d, p] = t
                    t2 = wpool.tile([128, 2, G4], bf16, tag=f"whh{d}{p}")
                    nc.sync.dma_start(
                        t2[:], whh[d, p].ap().rearrange("(k p) g -> p k g", p=128)
                    )
                    whh_sb[d, p] = t2
            whead_sb = {}
            for p in ("hi", "lo"):
                t = wpool.tile([128, 4, NH], bf16, tag=f"wh{p}")
                nc.sync.dma_start(t[:], whead[p].ap().rearrange("(k p) g -> p k g", p=128))
                whead_sb[p] = t
            bhead_sb = wpool.tile([NH, 1], f32, tag="bh")
            nc.sync.dma_start(bhead_sb[:], bhead[:])
            ident = wpool.tile([128, 128], f32, tag="ident")
            make_identity(nc, ident[:])

            # ---- phase A: xg[d][tok, 1024] = x @ w_ih[d].T + bias (aug row)
            def emit_a_tile(d, i):
                xt = apool.tile([128, 3, 128], f32, tag="xt")
                nc.sync.dma_start(
                    xt[:], xT.ap().rearrange("(k p) n -> p k n", p=128)[:, :, i * 128:(i + 1) * 128]
                )
                xhi = apool.tile([128, 3, 128], bf16, tag="xhi")
                xlo = apool.tile([128, 3, 128], bf16, tag="xlo")
                nc.vector.tensor_copy(xhi[:], xt[:])
                nc.vector.tensor_tensor(xlo[:], xt[:], xhi[:], AL.subtract)
                xgsb = apool.tile([128, G4], f32, tag="xgsb")
                for half in range(2):
                    ps = apsum.tile([128, 512], f32, tag="aps")
                    mm = 0
                    for k in range(3):
                        for ah, bh_ in (("hi", "hi"), ("hi", "lo"), ("lo", "hi")):
                            xop = xhi if ah == "hi" else xlo
                            nc.tensor.matmul(
                                ps[:],
                                xop[:, k, :],
                                wih_sb[d, bh_][:, k, half * 512:(half + 1) * 512],
                                start=(mm == 0),
                                stop=(mm == 8),
                            )
                            mm += 1
                    nc.scalar.activation(xgsb[:, half * 512:(half + 1) * 512], ps[:], AF.Copy)
                nc.sync.dma_start(xg[d][i * 128:(i + 1) * 128, :], xgsb[:])

            for i in range(NT):
                emit_a_tile("f", i)
                emit_a_tile("b", NT - 1 - i)

            # ---- phase B: recurrence. partition = (dir, batch): 0:64 f, 64:128 b
            hT_hi = hcpool.tile([128, 2, 128], bf16, tag="hThi")
            hT_lo = hcpool.tile([128, 2, 128], bf16, tag="hTlo")
            c_prev = hcpool.tile([128, 256], f32, tag="c")
            nc.vector.memset(hT_hi[:], 0.0)
            nc.vector.memset(hT_lo[:], 0.0)
            nc.vector.memset(c_prev[:], 0.0)

            for step in range(S):
                tf, tb = step, S - 1 - step
                # xg for this step: rows tf*64.. (dir f) and tb*64.. (dir b)
                xgt = bpool.tile([128, G4], f32, tag="xgt")
                nc.sync.dma_start(xgt[0:64, :], xg["f"][tf * 64:(tf + 1) * 64, :])
                nc.sync.dma_start(xgt[64:128, :], xg["b"][tb * 64:(tb + 1) * 64, :])

                ps = gpsum.tile([128, G4], f32, tag="gps")
                gates = bpool.tile([128, G4], f32, tag="gates")
                sig_if = bpool.tile([128, 512], f32, tag="sigif")
                tg = bpool.tile([128, 256], f32, tag="tg")
                so = bpool.tile([128, 256], f32, tag="so")
                for half in range(2):
                    for di, d in enumerate(("f", "b")):
                        out = ps[di * 64:(di + 1) * 64, half * 512:(half + 1) * 512]
                        mm = 0
                        for k in range(2):
                            for ah, bh_ in (("hi", "hi"), ("hi", "lo"), ("lo", "hi")):
                                hop = hT_hi if ah == "hi" else hT_lo
                                nc.tensor.matmul(
                                    out,
                                    hop[:, k, di * 64:(di + 1) * 64],
                                    whh_sb[d, bh_][:, k, half * 512:(half + 1) * 512],
                                    start=(mm == 0),
                                    stop=(mm == 5),
                                )
                                mm += 1
                    hsl = slice(half * 512, (half + 1) * 512)
                    nc.vector.tensor_tensor(gates[:, hsl], ps[:, hsl], xgt[:, hsl], AL.add)
                    if half == 0:
                        nc.scalar.activation(sig_if[:], gates[:, 0:512], AF.Sigmoid)
                    else:
                        nc.scalar.activation(tg[:], gates[:, 512:768], AF.Tanh)
                        nc.scalar.activation(so[:], gates[:, 768:1024], AF.Sigmoid)
                tmp = bpool.tile([128, 256], f32, tag="tmp")
                nc.vector.tensor_tensor(tmp[:], sig_if[:, 0:256], tg[:], AL.mult)
                c_new = hcpool.tile([128, 256], f32, tag="c")
                nc.vector.tensor_tensor(c_new[:], sig_if[:, 256:512], c_prev[:], AL.mult)
                nc.vector.tensor_tensor(c_new[:], c_new[:], tmp[:], AL.add)
                tcn = bpool.tile([128, 256], f32, tag="tcn")
                nc.scalar.activation(tcn[:], c_new[:], AF.Tanh)
                h_new = bpool.tile([128, 256], f32, tag="hnew")
                nc.vector.tensor_tensor(h_new[:], so[:], tcn[:], AL.mult)
                c_prev = c_new
                # transpose h [128(dir,b), 256(hdim)] -> [128(hdim), 2, 128(dir,b)]
                pt = tpsum.tile([128, 2, 128], f32, tag="pt")
                nc.tensor.transpose(pt[:, 0, :], h_new[:, 0:128], ident[:])
                nc.tensor.transpose(pt[:, 1, :], h_new[:, 128:256], ident[:])
                hT_hi = hcpool.tile([128, 2, 128], bf16, tag="hThi")
                hT_lo = hcpool.tile([128, 2, 128], bf16, tag="hTlo")
                nc.vector.tensor_copy(hT_hi[:], pt[:])
                nc.vector.tensor_tensor(hT_lo[:], pt[:], hT_hi[:], AL.subtract)
                for p, htile in (("hi", hT_hi), ("lo", hT_lo)):
                    dst = hsT["f", p].ap().rearrange("k p n -> p k n")
                    nc.sync.dma_start(dst[:, :, tf * 64:(tf + 1) * 64], htile[:, :, 0:64])
                    dst = hsT["b", p].ap().rearrange("k p n -> p k n")
                    nc.sync.dma_start(dst[:, :, tb * 64:(tb + 1) * 64], htile[:, :, 64:128])

            # ---- phase C: heads
            for c in range(NC_CH):
                sl = slice(c * 512, (c + 1) * 512)
                hks = {}
                for d in ("f", "b"):
                    for p in ("hi", "lo"):
                        t = cpool.tile([128, 2, 512], bf16, tag=f"hk{d}{p}")
                        nc.sync.dma_start(t[:], hsT[d, p].ap().rearrange("k p n -> p k n")[:, :, sl])
                        hks[d, p] = t
                ps = cpsum.tile([NH, 512], f32, tag="cps")
                mm = 0
                for di, d in enumerate(("f", "b")):
                    for k in range(2):
                        kh = 2 * di + k
                        for wp, hp in (("hi", "hi"), ("hi", "lo"), ("lo", "hi")):
                            nc.tensor.matmul(
                                ps[:],
                                whead_sb[wp][:, kh, :],
                                hks[d, hp][:, k, :],
                                start=(mm == 0),
                                stop=(mm == 11),
                            )
                            mm += 1
                osb = cpool.tile([NH, 512], f32, tag="osb")
                nc.scalar.activation(osb[:], ps[:], AF.Identity, bias=bhead_sb[:])
                nc.sync.dma_start(logitsT[:, sl], osb[:])

    return nc


def _get_nc():
    if "nc" not in _CACHE:
        nc = _build()
        nc.finalize()
        _CACHE["nc"] = nc
    return _CACHE["nc"]


def _prep_inputs(input_ids, w_ih_f, w_hh_f, b_ih_f, b_hh_f, w_ih_b, w_hh_b,
                 b_ih_b, b_hh_b, w_cls, b_cls, w_se, b_se):
    """Build per-core in_maps."""
    shared = {}
    for d, w_ih, w_hh, b_ih, b_hh in (
        ("f", w_ih_f, w_hh_f, b_ih_f, b_hh_f),
        ("b", w_ih_b, w_hh_b, b_ih_b, b_hh_b),
    ):
        wt = np.zeros((DP, G4), np.float32)
        wt[:D, :] = np.asarray(w_ih, np.float32).T
        wt[D, :] = np.asarray(b_ih, np.float32) + np.asarray(b_hh, np.float32)
        hi, lo = _split_hilo(wt)
        shared[f"wih_{d}_hi"], shared[f"wih_{d}_lo"] = hi, lo
        hi, lo = _split_hilo(np.asarray(w_hh, np.float32).T)
        shared[f"whh_{d}_hi"], shared[f"whh_{d}_lo"] = hi, lo
    whead = np.concatenate([np.asarray(w_cls, np.float32), np.asarray(w_se, np.float32)], 0).T
    hi, lo = _split_hilo(whead)
    shared["whead_hi"], shared["whead_lo"] = hi, lo
    shared["bhead"] = np.concatenate(
        [np.asarray(b_cls, np.float32), np.asarray(b_se, np.float32)]
    ).reshape(NH, 1)

    in_maps = []
    x = np.asarray(input_ids, np.float32)
    for c in range(NCORES):
        xs = x[c * BC:(c + 1) * BC]                    # [64, 128, 300]
        xt = np.zeros((DP, TOK), np.float32)
        xt[:D] = np.ascontiguousarray(xs.transpose(2, 1, 0)).reshape(D, TOK)
        xt[D] = 1.0
        m = dict(shared)
        m["xT"] = xt
        in_maps.append(m)
    return in_maps


# ---------------- host-side CRF / viterbi ----------------

def _crf_loss_np(em, tags, start, end, trans):
    # mask all ones; em [B,S,T] f32, tags [B,S] int
    Bn, Sn, T = em.shape
    em64 = em.astype(np.float64)
    tr64 = trans.astype(np.float64)
    bi = np.arange(Bn)
    em_sel = np.take_along_axis(em64, tags[..., None], axis=-1)[..., 0]
    tr_path = tr64[tags[:, :-1], tags[:, 1:]]
    num = start.astype(np.float64)[tags[:, 0]] + em_sel[:, 0] + (tr_path + em_sel[:, 1:]).sum(1)
    num = num + end.astype(np.float64)[tags[:, -1]]
    alpha = start.astype(np.float64)[None] + em64[:, 0]
    for t in range(1, Sn):
        cand = alpha[:, :, None] + tr64[None] + em64[:, t, None, :]
        m = cand.max(1)
        alpha = m + np.log(np.exp(cand - m[:, None, :]).sum(1))
    evec = end.astype(np.float64)[None]
    m = (alpha + evec).max(1)
    denom = m + np.log(np.exp(alpha + evec - m[:, None]).sum(1))
    return num - denom   # llh per sequence


def _viterbi_np(em, start, end, trans):
    Bn, Sn, T = em.shape
    em = em.astype(np.float32)
    score = start.astype(np.float32)[None] + em[:, 0]
    hist = np.zeros((Bn, Sn - 1, T), np.int32)
    for t in range(1, Sn):
        cand = score[:, :, None] + trans.astype(np.float32)[None] + em[:, t, None, :]
        hist[:, t - 1] = cand.argmax(1)
        score = cand.max(1)
    last = (score + end.astype(np.float32)[None]).argmax(1)
    tags = np.zeros((Bn, Sn), np.int64)
    tags[:, -1] = last
    cur = last
    bi = np.arange(Bn)
    for t in range(Sn - 2, -1, -1):
        cur = hist[bi, t, cur]
        tags[:, t] = cur
    return tags


def kernel(input_ids, attention_mask, labels, se_labels, w_ih_f, w_hh_f, b_ih_f,
           b_hh_f, w_ih_b, w_hh_b, b_ih_b, b_hh_b, w_cls, b_cls, w_se, b_se,
           start_t, end_t, trans_t, start_se, end_se, trans_se):
    from concourse.bass_utils import run_bass_kernel_spmd

    nc = _get_nc()
    in_maps = _prep_inputs(input_ids, w_ih_f, w_hh_f, b_ih_f, b_hh_f, w_ih_b,
                           w_hh_b, b_ih_b, b_hh_b, w_cls, b_cls, w_se, b_se)
    res = run_bass_kernel_spmd(nc, in_maps, list(range(NCORES)))
    outs = res.results

    logits = np.zeros((B, S, L), np.float32)
    se_logits = np.zeros((B, S, LSE), np.float32)
    for c in range(NCORES):
        lt = outs[c]["logitsT"].reshape(NH, S, BC)      # [14, t, b]
        blk = lt.transpose(2, 1, 0)                      # [b, t, 14]
        logits[c * BC:(c + 1) * BC] = blk[:, :, :L]
        se_logits[c * BC:(c + 1) * BC] = blk[:, :, L:]

    labels = np.asarray(labels).astype(np.int64)
    se_labels = np.asarray(se_labels).astype(np.int64)
    nmask = float(B * S)
    llh_t = _crf_loss_np(logits, labels, np.asarray(start_t), np.asarray(end_t), np.asarray(trans_t))
    llh_se = _crf_loss_np(se_logits, se_labels, np.asarray(start_se), np.asarray(end_se), np.asarray(trans_se))
    loss = np.float32(-(llh_t.sum() / nmask) - (llh_se.sum() / nmask))

    tags = _viterbi_np(logits, np.asarray(start_t), np.asarray(end_t), np.asarray(trans_t))
    se_tags = _viterbi_np(se_logits, np.asarray(start_se), np.asarray(end_se), np.asarray(trans_se))
    return (np.array(loss, np.float32), logits, se_logits,
            tags.astype(np.int32), se_tags.astype(np.int32))


# revision 9
# speedup vs baseline: 1.2750x; 1.2750x over previous
"""BiLSTM + two linear heads + two CRF losses + two Viterbi decodes on 8 TRN2 cores.

Device (per core, data-parallel over batch, 64 seqs/core):
  phase A: xg[d] = x @ w_ih[d].T + b  (hi/lo-split bf16 matmuls, fp32 accum)
  phase B: 128-step LSTM recurrence, fwd+bwd interleaved, both directions
           packed on the partition axis for the elementwise chain
  phase C: logitsT[14, tok] = [w_cls; w_se] @ [h_f; h_b]
Host: CRF forward (log-space, vectorized) + Viterbi + loss assembly from logits.
"""

import numpy as np

B, S, D, H, L, LSE = 512, 128, 300, 256, 9, 5
NCORES, BC = 8, 64
TOK = S * BC          # tokens per core, col index = t*BC + b
DP = 384              # D padded: 300 data rows + 1 ones row (bias) + zero pad
G4 = 4 * H            # 1024 gate dims
NH = L + LSE          # 14 combined head classes

_CACHE = {}


def _split_hilo(x):
    import ml_dtypes
    x = np.asarray(x, np.float32)
    hi = x.astype(ml_dtypes.bfloat16)
    lo = (x - hi.astype(np.float32)).astype(ml_dtypes.bfloat16)
    return hi, lo


def _build():
    from contextlib import ExitStack
    import concourse.bass as bass
    import concourse.tile as tile
    from concourse import bacc, mybir
    from concourse.masks import make_identity

    f32 = mybir.dt.float32
    bf16 = mybir.dt.bfloat16
    AL = mybir.AluOpType
    AF = mybir.ActivationFunctionType

    nc = bacc.Bacc()
    dp = nc.declare_dram_parameter
    xT = dp("xT", [DP, TOK], f32, isOutput=False)
    wih = {}
    whh = {}
    for d in ("f", "b"):
        for p in ("hi", "lo"):
            wih[d, p] = dp(f"wih_{d}_{p}", [DP, G4], bf16, isOutput=False)
            whh[d, p] = dp(f"whh_{d}_{p}", [H, G4], bf16, isOutput=False)
    whead = {p: dp(f"whead_{p}", [2 * H, NH], bf16, isOutput=False) for p in ("hi", "lo")}
    bhead = dp("bhead", [NH, 1], f32, isOutput=False)
    logitsT = dp("logitsT", [NH, TOK], f32, isOutput=True)

    # DRAM scratch
    xg = {d: nc.dram_tensor(f"xg_{d}", [TOK, G4], f32) for d in ("f", "b")}
    hsT = {}
    for d in ("f", "b"):
        for p in ("hi", "lo"):
            hsT[d, p] = nc.dram_tensor(f"hsT_{d}_{p}", [2, 128, TOK], bf16)

    NT = TOK // 128       # 64 token-tiles per direction in phase A
    NC_CH = TOK // 512    # 16 head chunks in phase C

    with tile.TileContext(nc) as tc:
        with ExitStack() as ctx:
            wpool = ctx.enter_context(tc.tile_pool(name="weights", bufs=1))
            apool = ctx.enter_context(tc.tile_pool(name="aphase", bufs=3))
            apsum = ctx.enter_context(tc.tile_pool(name="apsum", bufs=2, space="PSUM"))
            bpool = ctx.enter_context(tc.tile_pool(name="bphase", bufs=3))
            hcpool = ctx.enter_context(tc.tile_pool(name="hc", bufs=2))
            gpsum = ctx.enter_context(tc.tile_pool(name="gpsum", bufs=2, space="PSUM"))
            tpsum = ctx.enter_context(tc.tile_pool(name="tpsum", bufs=1, space="PSUM"))
            cpool = ctx.enter_context(tc.tile_pool(name="cphase", bufs=3))
            cpsum = ctx.enter_context(tc.tile_pool(name="cpsum", bufs=1, space="PSUM"))

            # ---- load weights once
            wih_sb = {}
            whh_sb = {}
            for d in ("f", "b"):
                for p in ("hi", "lo"):
                    t = wpool.tile([128, 3, G4], bf16, tag=f"wih{d}{p}")
                    nc.sync.dma_start(
                        t[:], wih[d, p].ap().rearrange("(k p) g -> p k g", p=128)
                    )
                    wih_sb[d, p] = t
                    t2 = wpool.tile([128, 2, G4], bf16, tag=f"whh{d}{p}")
                    nc.sync.dma_start(
                        t2[:], whh[d, p].ap().rearrange("(k p) g -> p k g", p=128)
                    )
                    whh_sb[d, p] = t2
            whead_sb = {}
            for p in ("hi", "lo"):
                t = wpool.tile([128, 4, NH], bf16, tag=f"wh{p}")
                nc.sync.dma_start(t[:], whead[p].ap().rearrange("(k p) g -> p k g", p=128))
                whead_sb[p] = t
            bhead_sb = wpool.tile([NH, 1], f32, tag="bh")
            nc.sync.dma_start(bhead_sb[:], bhead[:])
            ident = wpool.tile([128, 128], f32, tag="ident")
            make_identity(nc, ident[:])

            # ---- phase A: xg[d][tok, 1024] = x @ w_ih[d].T + bias (aug row)
            def emit_a_tile(d, i):
                xt = apool.tile([128, 3, 128], f32, tag="xt")
                nc.sync.dma_start(
                    xt[:], xT.ap().rearrange("(k p) n -> p k n", p=128)[:, :, i * 128:(i + 1) * 128]
                )
                xhi = apool.tile([128, 3, 128], bf16, tag="xhi")
                xlo = apool.tile([128, 3, 128], bf16, tag="xlo")
                nc.vector.tensor_copy(xhi[:], xt[:])
                nc.vector.tensor_tensor(xlo[:], xt[:], xhi[:], AL.subtract)
                xgsb = apool.tile([128, G4], f32, tag="xgsb")
                for half in range(2):
                    ps = apsum.tile([128, 512], f32, tag="aps")
                    mm = 0
                    for k in range(3):
                        for ah, bh_ in (("hi", "hi"), ("hi", "lo"), ("lo", "hi")):
                            xop = xhi if ah == "hi" else xlo
                            nc.tensor.matmul(
                                ps[:],
                                xop[:, k, :],
                                wih_sb[d, bh_][:, k, half * 512:(half + 1) * 512],
                                start=(mm == 0),
                                stop=(mm == 8),
                            )
                            mm += 1
                    nc.scalar.activation(xgsb[:, half * 512:(half + 1) * 512], ps[:], AF.Copy)
                nc.sync.dma_start(xg[d][i * 128:(i + 1) * 128, :], xgsb[:])

            for i in range(NT):
                emit_a_tile("f", i)
                emit_a_tile("b", NT - 1 - i)

            # ---- phase B: recurrence. partition = (dir, batch): 0:64 f, 64:128 b
            hT_hi = hcpool.tile([128, 2, 128], bf16, tag="hThi")
            hT_lo = hcpool.tile([128, 2, 128], bf16, tag="hTlo")
            c_prev = hcpool.tile([128, 256], f32, tag="c")
            nc.vector.memset(hT_hi[:], 0.0)
            nc.vector.memset(hT_lo[:], 0.0)
            nc.vector.memset(c_prev[:], 0.0)

            for step in range(S):
                tf, tb = step, S - 1 - step
                # xg for this step: rows tf*64.. (dir f) and tb*64.. (dir b)
                xgt = bpool.tile([128, G4], f32, tag="xgt")
                nc.sync.dma_start(xgt[0:64, :], xg["f"][tf * 64:(tf + 1) * 64, :])
                nc.sync.dma_start(xgt[64:128, :], xg["b"][tb * 64:(tb + 1) * 64, :])

                ps = gpsum.tile([128, G4], f32, tag="gps")
                gates = bpool.tile([128, G4], f32, tag="gates")
                sig_if = bpool.tile([128, 512], f32, tag="sigif")
                tg = bpool.tile([128, 256], f32, tag="tg")
                so = bpool.tile([128, 256], f32, tag="so")
                for half in range(2):
                    for di, d in enumerate(("f", "b")):
                        out = ps[di * 64:(di + 1) * 64, half * 512:(half + 1) * 512]
                        mm = 0
                        for k in range(2):
                            for ah, bh_ in (("hi", "hi"), ("hi", "lo"), ("lo", "hi")):
                                hop = hT_hi if ah == "hi" else hT_lo
                                nc.tensor.matmul(
                                    out,
                                    hop[:, k, di * 64:(di + 1) * 64],
                                    whh_sb[d, bh_][:, k, half * 512:(half + 1) * 512],
                                    start=(mm == 0),
                                    stop=(mm == 5),
                                )
                                mm += 1
                    hsl = slice(half * 512, (half + 1) * 512)
                    nc.vector.tensor_tensor(gates[:, hsl], ps[:, hsl], xgt[:, hsl], AL.add)
                    if half == 0:
                        nc.scalar.activation(sig_if[:], gates[:, 0:512], AF.Sigmoid)
                    else:
                        nc.scalar.activation(tg[:], gates[:, 512:768], AF.Tanh)
                        nc.scalar.activation(so[:], gates[:, 768:1024], AF.Sigmoid)
                tmp = bpool.tile([128, 256], f32, tag="tmp")
                nc.vector.tensor_tensor(tmp[:], sig_if[:, 0:256], tg[:], AL.mult)
                c_new = hcpool.tile([128, 256], f32, tag="c")
                nc.vector.tensor_tensor(c_new[:], sig_if[:, 256:512], c_prev[:], AL.mult)
                nc.vector.tensor_tensor(c_new[:], c_new[:], tmp[:], AL.add)
                tcn = bpool.tile([128, 256], f32, tag="tcn")
                nc.scalar.activation(tcn[:], c_new[:], AF.Tanh)
                h_new = bpool.tile([128, 256], f32, tag="hnew")
                nc.vector.tensor_tensor(h_new[:], so[:], tcn[:], AL.mult)
                c_prev = c_new
                # transpose h [128(dir,b), 256(hdim)] -> [128(hdim), 2, 128(dir,b)]
                pt = tpsum.tile([128, 2, 128], f32, tag="pt")
                nc.tensor.transpose(pt[:, 0, :], h_new[:, 0:128], ident[:])
                nc.tensor.transpose(pt[:, 1, :], h_new[:, 128:256], ident[:])
                hT_hi = hcpool.tile([128, 2, 128], bf16, tag="hThi")
                hT_lo = hcpool.tile([128, 2, 128], bf16, tag="hTlo")
                nc.vector.tensor_copy(hT_hi[:], pt[:])
                nc.vector.tensor_tensor(hT_lo[:], pt[:], hT_hi[:], AL.subtract)
                for p, htile in (("hi", hT_hi), ("lo", hT_lo)):
                    dst = hsT["f", p].ap().rearrange("k p n -> p k n")
                    nc.sync.dma_start(dst[:, :, tf * 64:(tf + 1) * 64], htile[:, :, 0:64])
                    dst = hsT["b", p].ap().rearrange("k p n -> p k n")
                    nc.sync.dma_start(dst[:, :, tb * 64:(tb + 1) * 64], htile[:, :, 64:128])

            # ---- phase C: heads
            for c in range(NC_CH):
                sl = slice(c * 512, (c + 1) * 512)
                hks = {}
                for d in ("f", "b"):
                    for p in ("hi", "lo"):
                        t = cpool.tile([128, 2, 512], bf16, tag=f"hk{d}{p}")
                        nc.sync.dma_start(t[:], hsT[d, p].ap().rearrange("k p n -> p k n")[:, :, sl])
                        hks[d, p] = t
                ps = cpsum.tile([NH, 512], f32, tag="cps")
                mm = 0
                for di, d in enumerate(("f", "b")):
                    for k in range(2):
                        kh = 2 * di + k
                        for wp, hp in (("hi", "hi"), ("hi", "lo"), ("lo", "hi")):
                            nc.tensor.matmul(
                                ps[:],
                                whead_sb[wp][:, kh, :],
                                hks[d, hp][:, k, :],
                                start=(mm == 0),
                                stop=(mm == 11),
                            )
                            mm += 1
                osb = cpool.tile([NH, 512], f32, tag="osb")
                nc.scalar.activation(osb[:], ps[:], AF.Identity, bias=bhead_sb[:])
                nc.sync.dma_start(logitsT[:, sl], osb[:])

    return nc


def _get_nc():
    if "nc" not in _CACHE:
        nc = _build()
        nc.finalize()
        _CACHE["nc"] = nc
    return _CACHE["nc"]


def _prep_inputs(input_ids, w_ih_f, w_hh_f, b_ih_f, b_hh_f, w_ih_b, w_hh_b,
                 b_ih_b, b_hh_b, w_cls, b_cls, w_se, b_se):
    """Build per-core in_maps."""
    shared = {}
    for d, w_ih, w_hh, b_ih, b_hh in (
        ("f", w_ih_f, w_hh_f, b_ih_f, b_hh_f),
        ("b", w_ih_b, w_hh_b, b_ih_b, b_hh_b),
    ):
        wt = np.zeros((DP, G4), np.float32)
        wt[:D, :] = np.asarray(w_ih, np.float32).T
        wt[D, :] = np.asarray(b_ih, np.float32) + np.asarray(b_hh, np.float32)
        hi, lo = _split_hilo(wt)
        shared[f"wih_{d}_hi"], shared[f"wih_{d}_lo"] = hi, lo
        hi, lo = _split_hilo(np.asarray(w_hh, np.float32).T)
        shared[f"whh_{d}_hi"], shared[f"whh_{d}_lo"] = hi, lo
    whead = np.concatenate([np.asarray(w_cls, np.float32), np.asarray(w_se, np.float32)], 0).T
    hi, lo = _split_hilo(whead)
    shared["whead_hi"], shared["whead_lo"] = hi, lo
    shared["bhead"] = np.concatenate(
        [np.asarray(b_cls, np.float32), np.asarray(b_se, np.float32)]
    ).reshape(NH, 1)

    in_maps = []
    x = np.asarray(input_ids, np.float32)
    for c in range(NCORES):
        xs = x[c * BC:(c + 1) * BC]                    # [64, 128, 300]
        xt = np.zeros((DP, TOK), np.float32)
        xt[:D] = np.ascontiguousarray(xs.transpose(2, 1, 0)).reshape(D, TOK)
        xt[D] = 1.0
        m = dict(shared)
        m["xT"] = xt
        in_maps.append(m)
    return in_maps


# ---------------- host-side CRF / viterbi ----------------

def _crf_loss_np(em, tags, mask, start, end, trans):
    # torchcrf log-likelihood per sequence; em [B,S,T], tags [B,S], mask [B,S]
    Bn, Sn, T = em.shape
    em64 = em.astype(np.float64)
    tr64 = trans.astype(np.float64)
    mf = mask.astype(np.float64)
    bi = np.arange(Bn)
    em_sel = np.take_along_axis(em64, tags[..., None], axis=-1)[..., 0]
    tr_path = tr64[tags[:, :-1], tags[:, 1:]]
    num = start.astype(np.float64)[tags[:, 0]] + em_sel[:, 0] + (
        (tr_path + em_sel[:, 1:]) * mf[:, 1:]).sum(1)
    last_idx = mask.astype(np.int64).sum(1) - 1
    num = num + end.astype(np.float64)[tags[bi, last_idx]]
    alpha = start.astype(np.float64)[None] + em64[:, 0]
    for t in range(1, Sn):
        cand = alpha[:, :, None] + tr64[None] + em64[:, t, None, :]
        m = cand.max(1)
        new_a = m + np.log(np.exp(cand - m[:, None, :]).sum(1))
        alpha = np.where(mf[:, t:t + 1] > 0, new_a, alpha)
    evec = end.astype(np.float64)[None]
    m = (alpha + evec).max(1)
    denom = m + np.log(np.exp(alpha + evec - m[:, None]).sum(1))
    return num - denom   # llh per sequence


def _viterbi_np(em, mask, start, end, trans):
    Bn, Sn, T = em.shape
    em = em.astype(np.float32)
    mb = mask.astype(bool)
    score = start.astype(np.float32)[None] + em[:, 0]
    hist = np.zeros((Bn, Sn - 1, T), np.int32)
    for t in range(1, Sn):
        cand = score[:, :, None] + trans.astype(np.float32)[None] + em[:, t, None, :]
        hist[:, t - 1] = cand.argmax(1)
        score = np.where(mb[:, t:t + 1], cand.max(1), score)
    last = (score + end.astype(np.float32)[None]).argmax(1)
    tags = np.zeros((Bn, Sn), np.int64)
    tags[:, -1] = last
    cur = last
    bi = np.arange(Bn)
    for t in range(Sn - 2, -1, -1):
        cur = np.where(mb[:, t + 1], hist[bi, t, cur], cur)
        tags[:, t] = cur
    return tags


def kernel(input_ids, attention_mask, labels, se_labels, w_ih_f, w_hh_f, b_ih_f,
           b_hh_f, w_ih_b, w_hh_b, b_ih_b, b_hh_b, w_cls, b_cls, w_se, b_se,
           start_t, end_t, trans_t, start_se, end_se, trans_se):
    from concourse.bass_utils import run_bass_kernel_spmd

    nc = _get_nc()
    in_maps = _prep_inputs(input_ids, w_ih_f, w_hh_f, b_ih_f, b_hh_f, w_ih_b,
                           w_hh_b, b_ih_b, b_hh_b, w_cls, b_cls, w_se, b_se)
    res = run_bass_kernel_spmd(nc, in_maps, list(range(NCORES)))
    outs = res.results

    logits = np.zeros((B, S, L), np.float32)
    se_logits = np.zeros((B, S, LSE), np.float32)
    for c in range(NCORES):
        lt = outs[c]["logitsT"].reshape(NH, S, BC)      # [14, t, b]
        blk = lt.transpose(2, 1, 0)                      # [b, t, 14]
        logits[c * BC:(c + 1) * BC] = blk[:, :, :L]
        se_logits[c * BC:(c + 1) * BC] = blk[:, :, L:]

    labels = np.asarray(labels).astype(np.int64)
    se_labels = np.asarray(se_labels).astype(np.int64)
    amask = np.asarray(attention_mask)
    nmask = float(amask.astype(np.float64).sum())
    llh_t = _crf_loss_np(logits, labels, amask, np.asarray(start_t), np.asarray(end_t), np.asarray(trans_t))
    llh_se = _crf_loss_np(se_logits, se_labels, amask, np.asarray(start_se), np.asarray(end_se), np.asarray(trans_se))
    loss = np.float32(-(llh_t.sum() / nmask) - (llh_se.sum() / nmask))

    tags = _viterbi_np(logits, amask, np.asarray(start_t), np.asarray(end_t), np.asarray(trans_t))
    se_tags = _viterbi_np(se_logits, amask, np.asarray(start_se), np.asarray(end_se), np.asarray(trans_se))
    return (np.array(loss, np.float32), logits, se_logits,
            tags.astype(np.int32), se_tags.astype(np.int32))
